# revision 30
# baseline (speedup 1.0000x reference)
"""Head-sharded causal self-attention (value-residual + RMSNorm + RoPE) for 8 TRN2 cores.

Sharding: 2 heads per core (tensor parallel). Each core computes q/k/v for its
128 dims, full causal attention for its heads, and a partial c_proj output;
the host sums the 8 partial [T, D] outputs (the TP all-reduce).

v3 schedule:
  - Scalar (ACT) engine needs exactly 2 table loads: phase A runs only
    COPY+LN (natural_log table) for the RMSNorm stats of all chunks;
    phase C runs only EXP. Squares for the stats happen on DVE from bf16.
  - lambda-mix of vi folded into an augmented V GEMM (x|vi contraction).
  - causal mask applied additively in PSUM via a -240*I x tril matmul.
  - softmax denominator via DVE reciprocal_approx_fast straight off PSUM.
  - bf16 DRAM inputs + bf16 partial-y output; input DMA spread over
    several engine queues so phase A is not gated on one DMA queue.
  - attention inner loop software-pipelined (QK(j+1) issued before PV(j));
    c_proj matmuls of chunk c-1 drained into tensor-idle slots of chunk c;
    PSUM->SBUF casts split between vector and gpsimd.
"""
import os
import sys

sys.path.insert(0, "/opt/trn_rl_repo")

import numpy as np
import ml_dtypes

import concourse.bacc as bacc
import concourse.tile as tile
import concourse.bass as bass
from concourse import mybir
from concourse.bass_utils import run_bass_kernel_spmd

N_CORES = 8
T, D, H, HD = 2048, 1024, 16, 64
HS = H // N_CORES            # 2 heads per core
J = HS * HD                  # 128
NT = T // 128                # 16 s-tiles
NCH = T // 512               # 4 chunks
KT = D // 128                # 8 contraction tiles for q/k
KV = KT + 1                  # 9 for the augmented v GEMM (x | vi)
F32 = mybir.dt.float32
BF16 = mybir.dt.bfloat16
AF = mybir.ActivationFunctionType
OP = mybir.AluOpType
EPS = float(np.finfo(np.float32).eps)
BF = ml_dtypes.bfloat16


def build_nc():
    nc = bacc.Bacc("TRN2", target_bir_lowering=False, debug=False,
                   num_devices=N_CORES)

    # host pre-transposed to partition-major so every DMA is contiguous
    # per partition (large descriptors, near-peak DMA rate)
    xaT = nc.dram_tensor("xaT", [128, NCH, KT, 512], BF16, kind="ExternalInput")
    viT = nc.dram_tensor("viT", [128, NT, J], BF16, kind="ExternalInput")
    wqT = nc.dram_tensor("wqT", [128, KT, J], BF16, kind="ExternalInput")
    wkT = nc.dram_tensor("wkT", [128, KT, J], BF16, kind="ExternalInput")
    wvT = nc.dram_tensor("wvT", [128, KT, J], BF16, kind="ExternalInput")
    wpT = nc.dram_tensor("wpT", [J, D], BF16, kind="ExternalInput")
    Ct = nc.dram_tensor("Ct", [J, T], BF16, kind="ExternalInput")
    St = nc.dram_tensor("St", [J, T], BF16, kind="ExternalInput")
    o2r = nc.dram_tensor("o2r", [128, 128], BF16, kind="ExternalInput")
    prm = nc.dram_tensor("prm", [128, 128], BF16, kind="ExternalInput")
    p64 = nc.dram_tensor("p64", [128, 128], BF16, kind="ExternalInput")
    mI = nc.dram_tensor("mI", [128, 128], BF16, kind="ExternalInput")
    tlo = nc.dram_tensor("tlo", [128, 128], BF16, kind="ExternalInput")
    y = nc.dram_tensor("y", [T, D], BF16, kind="ExternalOutput")

    with tile.TileContext(nc) as tc:
        with tc.tile_pool(name="persist", bufs=1) as pp:
            # ---- persistent loads, spread across engine DMA queues ----
            # chunk-0-critical loads first; spread across the three DMA-capable
            # queues (sync, gpsimd, scalar) so no queue serializes phase A
            xa = pp.tile([128, KT, T], BF16, tag="xa")
            nc.sync.dma_start(out=xa[:, 0:4, 0:512], in_=xaT[:, 0, 0:4, :])
            nc.gpsimd.dma_start(out=xa[:, 4:KT, 0:512], in_=xaT[:, 0, 4:KT, :])
            wq = pp.tile([128, KT, J], BF16, tag="wq")
            nc.scalar.dma_start(out=wq, in_=wqT[:, :, :])
            wk_ = pp.tile([128, KT, J], BF16, tag="wk")
            nc.scalar.dma_start(out=wk_, in_=wkT[:, :, :])
            wv = pp.tile([128, KT, J], BF16, tag="wv")
            nc.scalar.dma_start(out=wv, in_=wvT[:, :, :])
            o2r_sb = pp.tile([128, 128], BF16, tag="o2r")
            nc.scalar.dma_start(out=o2r_sb, in_=o2r[:, :])
            prm_sb = pp.tile([128, 128], BF16, tag="prm")
            nc.scalar.dma_start(out=prm_sb, in_=prm[:, :])
            nc.gpsimd.dma_start(out=xa[:, :, 512:1024], in_=xaT[:, 1, :, :])
            vi_sb = pp.tile([128, NT, J], BF16, tag="vi")
            nc.gpsimd.dma_start(out=vi_sb, in_=viT[:, :, :])
            csb = pp.tile([J, T], BF16, tag="csb")
            nc.sync.dma_start(out=csb, in_=Ct[:, :])
            ssb = pp.tile([J, T], BF16, tag="ssb")
            nc.sync.dma_start(out=ssb, in_=St[:, :])
            for c in (2, 3):
                tsl = slice(512 * c, 512 * (c + 1))
                nc.sync.dma_start(out=xa[:, :, tsl], in_=xaT[:, c, :, :])
            mI_sb = pp.tile([128, 128], BF16, tag="mI")
            nc.scalar.dma_start(out=mI_sb, in_=mI[:, :])
            tlo_sb = pp.tile([128, 128], BF16, tag="tlo")
            nc.scalar.dma_start(out=tlo_sb, in_=tlo[:, :])
            p64_sb = pp.tile([128, 128], BF16, tag="p64")
            nc.scalar.dma_start(out=p64_sb, in_=p64[:, :])
            wp = pp.tile([128, D], BF16, tag="wp")
            nc.scalar.dma_start(out=wp, in_=wpT[:, :])
            eps_sb = pp.tile([128, 1], F32, tag="eps")
            nc.vector.memset(eps_sb, EPS)

            # v_aug: [v_h0 | ones64 | ones64 | v_h1] per s-tile
            vaug = pp.tile([128, NT, 4, HD], BF16, tag="vaug")
            nc.gpsimd.memset(vaug[:, :, 1:3, :], 1.0)

            qh = pp.tile([J, T], BF16, tag="qh")      # roped raw q
            kh = pp.tile([J, T], BF16, tag="kh")      # roped raw k
            lnqk = pp.tile([128, 2, T], F32, tag="lnqk")  # ln(ms+eps): q,k
            rqk = pp.tile([128, 2, T], BF16, tag="rqk")   # rsqrt factors
            qhn = pp.tile([J, T], BF16, tag="qhn")    # normalized roped q
            khn = pp.tile([J, T], BF16, tag="khn")

            # ================= phase A =================
            # q/k/v GEMMs, stats + raw rope. Scalar: COPY + LN only.
            with (
                tc.tile_pool(name="pq", bufs=1, space="PSUM") as pq,
                tc.tile_pool(name="pk", bufs=1, space="PSUM") as pk,
                tc.tile_pool(name="pv", bufs=2, space="PSUM") as pv,
                tc.tile_pool(name="pms", bufs=1, space="PSUM") as pms,
                tc.tile_pool(name="pr", bufs=2, space="PSUM") as pr,
                tc.tile_pool(name="aw", bufs=2) as aw,
            ):
                for c in range(NCH):
                    tsl = slice(512 * c, 512 * (c + 1))
                    q_ps = pq.tile([128, 512], F32, tag="q")
                    for kk in range(KT):
                        nc.tensor.matmul(q_ps, wq[:, kk, :], xa[:, kk, tsl],
                                         start=(kk == 0), stop=(kk == KT - 1))
                    k_ps = pk.tile([128, 512], F32, tag="k")
                    for kk in range(KT):
                        nc.tensor.matmul(k_ps, wk_[:, kk, :], xa[:, kk, tsl],
                                         start=(kk == 0), stop=(kk == KT - 1))
                    # raw copies + squares on DVE: the ACT-table pass pins
                    # COPY to the exp table, so scalar must do ONLY Ln here
                    qraw = aw.tile([128, 512], BF16, tag="qraw")
                    nc.vector.tensor_copy(qraw, q_ps)
                    kraw = aw.tile([128, 512], BF16, tag="kraw")
                    nc.vector.tensor_copy(kraw, k_ps)
                    q2 = aw.tile([128, 512], BF16, tag="q2")
                    nc.vector.tensor_tensor(q2, qraw, qraw, OP.mult)
                    k2 = aw.tile([128, 512], BF16, tag="k2")
                    nc.vector.tensor_tensor(k2, kraw, kraw, OP.mult)
                    # mean-square stats -> single Ln for q and k
                    msqk = pms.tile([128, 2, 512], F32, tag="ms")
                    nc.tensor.matmul(msqk[:, 0, :], o2r_sb, q2,
                                     start=True, stop=True)
                    nc.tensor.matmul(msqk[:, 1, :], o2r_sb, k2,
                                     start=True, stop=True)
                    nc.scalar.activation(lnqk[:, :, tsl], msqk, AF.Ln,
                                         bias=eps_sb, scale=1.0 / HD)
                    # raw rope: qh = qraw*C + (P qraw)*S  (gpsimd + vector)
                    qs_ps = pr.tile([128, 512], F32, tag="rot")
                    nc.tensor.matmul(qs_ps, prm_sb, qraw, start=True, stop=True)
                    t1 = aw.tile([128, 512], BF16, tag="t1")
                    nc.gpsimd.tensor_mul(t1, qraw, csb[:, tsl])
                    t2 = aw.tile([128, 512], BF16, tag="t2")
                    nc.vector.tensor_tensor(t2, qs_ps, ssb[:, tsl], OP.mult)
                    nc.gpsimd.tensor_add(qh[:, tsl], t1, t2)
                    ks_ps = pr.tile([128, 512], F32, tag="rot")
                    nc.tensor.matmul(ks_ps, prm_sb, kraw, start=True, stop=True)
                    t3 = aw.tile([128, 512], BF16, tag="t3")
                    nc.gpsimd.tensor_mul(t3, kraw, csb[:, tsl])
                    t4 = aw.tile([128, 512], BF16, tag="t4")
                    nc.vector.tensor_tensor(t4, ks_ps, ssb[:, tsl], OP.mult)
                    nc.gpsimd.tensor_add(kh[:, tsl], t3, t4)

                # v GEMMs last: their tensor work overlaps the exp-table
                # switch + rsqrt exps that gate the attention phase
                for st in range(NT):
                    v_ps = pv.tile([128, 128], F32, tag="v")
                    for kk in range(KT):
                        nc.tensor.matmul(
                            v_ps,
                            xa[:, kk, 128 * st:128 * (st + 1)],
                            wv[:, kk, :],
                            start=(kk == 0), stop=(kk == KT - 1))
                    # value residual: v + lambda1*vi, straight into vaug
                    nc.vector.tensor_tensor(
                        vaug[:, st, 0:4:3, :],
                        v_ps.rearrange("p (h d) -> p h d", h=2),
                        vi_sb[:, st, :].rearrange("p (h d) -> p h d", h=2),
                        OP.add)

            # ================= phase C =================
            # per-chunk: normalize prologue (exp table) + attention + c_proj.
            with (
                tc.tile_pool(name="psc", bufs=2, space="PSUM") as psc,
                tc.tile_pool(name="pz", bufs=1, space="PSUM") as pz,
                tc.tile_pool(name="py", bufs=2, space="PSUM") as py,
                tc.tile_pool(name="at", bufs=3) as at,
                tc.tile_pool(name="zw", bufs=2) as zw,
                tc.tile_pool(name="yo", bufs=2) as yo,
            ):
                deferred = []

                def drain(n):
                    for _ in range(min(n, len(deferred))):
                        deferred.pop(0)()

                # one Exp per q/k over ALL chunks: each depends on all four
                # Lns, which pins the table switch to a single point no
                # matter how the scheduler reorders the scalar queue
                nc.scalar.activation(rqk[:, 1, :], lnqk[:, 1, :], AF.Exp,
                                     bias=0.0, scale=-0.5)
                nc.scalar.activation(rqk[:, 0, :], lnqk[:, 0, :], AF.Exp,
                                     bias=0.0, scale=-0.5)

                def prologue(c):
                    # normalized q/k for chunk c
                    tsl = slice(512 * c, 512 * (c + 1))
                    nc.vector.tensor_tensor(khn[:, tsl], kh[:, tsl],
                                            rqk[:, 1, tsl], OP.mult)
                    nc.vector.tensor_tensor(qhn[:, tsl], qh[:, tsl],
                                            rqk[:, 0, tsl], OP.mult)

                prologue(0)
                # last 512-chunk split into two 256 halves: the first half's
                # softmax-denominator + c_proj overlaps the second half's
                # attention, shrinking the end-of-kernel serial tail
                segments = [(0, 512), (512, 512), (1024, 512),
                            (1536, 256), (1792, 256)]
                for si, (t0, tw) in enumerate(segments):
                    n_st = (t0 + tw) // 128

                    zt2 = pz.tile([128, HS, 512], F32, tag="zt")
                    sc_t = {}
                    aT_t = {}

                    def emit_qk(j, t0=t0, tw=tw, sc_t=sc_t):
                        loc0 = max(0, 128 * j - t0)
                        is_diag = 128 * j >= t0
                        sch = psc.tile([128, HS, 512], F32, tag="sc")
                        for h in range(HS):
                            nc.tensor.matmul(
                                sch[:, h, loc0:tw],
                                khn[64 * h:64 * (h + 1),
                                    128 * j:128 * (j + 1)],
                                qhn[64 * h:64 * (h + 1), t0 + loc0:t0 + tw],
                                start=True, stop=not is_diag)
                            if is_diag:
                                # diagonal s-tile: add -240 upper-tri mask
                                nc.tensor.matmul(
                                    sch[:, h, loc0:loc0 + 128], mI_sb, tlo_sb,
                                    start=False, stop=True)
                        sc_t[j] = sch

                    def emit_exp(j, t0=t0, tw=tw, sc_t=sc_t, aT_t=aT_t):
                        loc0 = max(0, 128 * j - t0)
                        aT = at.tile([128, HS, 512], BF16, tag="aT")
                        sch = sc_t.pop(j)
                        nc.scalar.activation(
                            aT[:, :, loc0:tw], sch[:, :, loc0:tw],
                            AF.Exp, bias=0.0, scale=1.0 / 8.0)
                        aT_t[j] = aT

                    def emit_pv(j, t0=t0, tw=tw, n_st=n_st, zt2=zt2,
                                aT_t=aT_t):
                        loc0 = max(0, 128 * j - t0)
                        aT = aT_t.pop(j)
                        for h in range(HS):
                            nc.tensor.matmul(
                                zt2[:, h, loc0:tw],
                                vaug[:, j, 2 * h:2 * h + 2, :],
                                aT[:, h, loc0:tw],
                                start=(j == 0), stop=(j == n_st - 1))

                    # software-pipelined attention loop
                    emit_qk(0)
                    for j in range(n_st):
                        emit_exp(j)
                        if j + 1 < n_st:
                            emit_qk(j + 1)
                        drain(2)
                        emit_pv(j)

                    # prefetch the next 512-chunk's normalize prologue so its
                    # scalar/vector work overlaps this segment's epilogue
                    nxt = (t0 + tw) // 512
                    if t0 + tw == 512 * nxt and nxt < NCH:
                        prologue(nxt)

                    # epilogue: softmax denominator + c_proj (deferred so its
                    # tensor work fills gaps of the next segment's loop)
                    zzb = zw.tile([128, 512], BF16, tag="zzb")
                    nc.vector.tensor_copy(zzb[0:64, :tw], zt2[0:64, 1, :tw])
                    nc.vector.tensor_copy(zzb[64:128, :tw],
                                          zt2[64:128, 0, :tw])
                    ysb = yo.tile([128, 4, D], BF16, tag="ysb")
                    zn_box = {}

                    def zn_thunk(tw=tw, zt2=zt2, zzb=zzb, zn_box=zn_box):
                        zsw_ps = py.tile([128, 512], F32, tag="y")
                        nc.tensor.matmul(zsw_ps[:, :tw], p64_sb, zzb[:, :tw],
                                         start=True, stop=True)
                        rzf = zw.tile([128, 512], F32, tag="rzf")
                        nc.vector.reciprocal_approx_fast(out=rzf[:, :tw],
                                                         in_=zsw_ps[:, :tw])
                        ztn = zw.tile([128, 512], BF16, tag="ztn")
                        nc.vector.tensor_tensor(ztn[0:64, :tw],
                                                zt2[0:64, 0, :tw],
                                                rzf[0:64, :tw], OP.mult)
                        nc.vector.tensor_tensor(ztn[64:128, :tw],
                                                zt2[64:128, 1, :tw],
                                                rzf[64:128, :tw], OP.mult)
                        zn_box["ztn"] = ztn

                    deferred.append(zn_thunk)

                    def cproj_thunk(ti, ysb=ysb, zn_box=zn_box):
                        def go():
                            ztn = zn_box["ztn"]
                            for oc in range(2):
                                y_ps = py.tile([128, 512], F32, tag="y")
                                nc.tensor.matmul(
                                    y_ps,
                                    ztn[:, 128 * ti:128 * (ti + 1)],
                                    wp[:, 512 * oc:512 * (oc + 1)],
                                    start=True, stop=True)
                                nc.vector.tensor_copy(
                                    ysb[:, ti, 512 * oc:512 * (oc + 1)],
                                    y_ps)
                        return go

                    def dma_thunk(ti, t0=t0, si=si, ysb=ysb):
                        def go():
                            rows = slice(t0 + 128 * ti, t0 + 128 * (ti + 1))
                            engs = ([nc.sync, nc.scalar]
                                    if si == len(segments) - 1 else
                                    [nc.sync, nc.gpsimd, nc.sync, nc.gpsimd])
                            engs[ti % len(engs)].dma_start(out=y[rows, :],
                                                           in_=ysb[:, ti, :])
                        return go

                    for ti in range(tw // 128):
                        deferred.append(cproj_thunk(ti))
                        deferred.append(dma_thunk(ti))

                drain(len(deferred))

    nc.finalize()
    return nc


def _host_prep(x, vi, Wq, Wk, Wv, Wproj, lambdas):
    x = np.asarray(x, np.float32)[0]
    vi = np.asarray(vi, np.float32)[0]
    Wq, Wk, Wv = (np.asarray(a, np.float32) for a in (Wq, Wk, Wv))
    Wp = np.asarray(Wproj, np.float32)
    lam = np.asarray(lambdas, np.float32)

    xT = np.ascontiguousarray(x.T)
    quarter = HD // 4
    inv_freq = (1.0 / 1024.0) ** np.linspace(0.0, 1.0, quarter, dtype=np.float32)
    inv_freq = np.concatenate([inv_freq, np.zeros(quarter, np.float32)])
    th = np.arange(T, dtype=np.float32)[:, None] * inv_freq[None, :]
    cos, sin = np.cos(th).astype(np.float32), np.sin(th).astype(np.float32)
    C = np.zeros((J, T), np.float32)
    S = np.zeros((J, T), np.float32)
    for h in range(HS):
        C[h * 64:h * 64 + 32] = cos.T[:32]
        C[h * 64 + 32:h * 64 + 64] = cos.T[:32]
        S[h * 64:h * 64 + 32] = sin.T[:32]
        S[h * 64 + 32:h * 64 + 64] = -sin.T[:32]
    o2r = np.zeros((128, 128), np.float32)
    o2r[0:64, 0:64] = 1.0
    o2r[64:128, 64:128] = 1.0
    prm = np.zeros((128, 128), np.float32)
    for i in range(128):
        src = i + 32 if (i % 64) < 32 else i - 32
        prm[src, i] = 1.0
    p64 = np.zeros((128, 128), np.float32)
    for i in range(128):
        p64[(i + 64) % 128, i] = 1.0
    mI = -240.0 * np.eye(128, dtype=np.float32)
    tlo = np.tril(np.ones((128, 128), np.float32), -1)

    bf = lambda a: np.ascontiguousarray(a).astype(BF)
    # (k p) m  ->  p k m   (partition-major for contiguous DMA)
    pmaj = lambda a: np.ascontiguousarray(
        a.reshape(-1, 128, a.shape[-1]).transpose(1, 0, 2))

    # (k p) (nch t)  ->  p nch k t
    xa4 = bf(xT.reshape(KT, 128, NCH, 512).transpose(1, 2, 0, 3))
    in_maps = []
    for c in range(N_CORES):
        rows = slice(J * c, J * (c + 1))
        # lambda1-scaled vi in [t%128, s-tile, j] layout
        vi4 = (lam[1] * vi[:, rows]).reshape(NT, 128, J).transpose(1, 0, 2)
        in_maps.append({
            "xaT": xa4,
            "viT": bf(vi4),
            "wqT": bf(pmaj(Wq[rows, :].T)),
            "wkT": bf(pmaj(Wk[rows, :].T)),
            "wvT": bf(pmaj((lam[0] * Wv[rows, :]).T)),
            "wpT": bf(Wp[:, rows].T),
            "Ct": bf(C), "St": bf(S),
            "o2r": bf(o2r), "prm": bf(prm), "p64": bf(p64),
            "mI": bf(mI), "tlo": bf(tlo),
        })
    return in_maps


_NC = None


def kernel(x, vi, Wq, Wk, Wv, Wproj, lambdas):
    global _NC
    if _NC is None:
        _NC = build_nc()
    in_maps = _host_prep(x, vi, Wq, Wk, Wv, Wproj, lambdas)
    trace = bool(int(os.environ.get("KERNEL_TRACE", "0")))
    res = run_bass_kernel_spmd(_NC, in_maps, core_ids=list(range(N_CORES)),
                               trace=trace)
    if trace and res.exec_time_ns is not None:
        print(f"HW exec time: {res.exec_time_ns} ns")
    out = np.zeros((T, D), np.float32)
    for c in range(N_CORES):
        out += res.results[c]["y"].astype(np.float32)
    return out.reshape(1, T, D)
